# revision 4
# baseline (speedup 1.0000x reference)
"""Trainium2 Bass kernel for nn_DiverseRegDCConv2d.

Per-sample dynamic 3x3 conv: filters are generated per sample from an
8-column weight bank (wgen[b] = se[b] @ bank.T), then applied as a
standard 256->256 conv on 28x28 with padding 1.

Sharding (8 cores): 4 batch-groups x 2 out-channel halves. Each core
handles 8 samples x 128 out channels; the weight bank half it needs is
replicated across the 4 batch-groups. No cross-device communication.

Numerics: the conv runs on fp8e4 (e4m3) operands in DoubleRow perf
mode (K=256 per matmul, 0.5 cycles per output column) with a 3-term
residual split that recovers ~fp16 accuracy:

    out = (w8 (.) x8  +  w8 (.) dx8  +  dwq (.) x8) / 16

where the filters are generated on device at 16x scale (se pre-scaled
on host), w8 = fp8(W), dwq = fp8(W - w8), and the activations are
split on host as x8 = fp8(x), dx8 = fp8(x - x8). End-to-end relative
error vs the fp32 reference is ~1.5e-3.

Conv layout trick: activations are stored width-29 row-flattened
(left zero-pad column only; the right pad of row r aliases the left
zero of row r+1), so every 3x3 shifted window is a single contiguous
406-element run and the DoubleRow moving AP stays 3-D. Each PSUM row
has one discarded halo column (29 vs 28).

Filter generation stays on device: the bank half is pre-arranged on
the host into 128x128 stationary tiles whose partition axis is (n, g)
with n = bank column (8) and g = 16 (k,o)-blocks; the streaming
operand is a block-diagonal arrangement of 16*inputs_se. Evacuation is
split into three passes so PSUM turns around fast and the slow fp8
ops run SBUF->SBUF where GPSIMD can help: (1) Act/DVE copy PSUM->W16
scratch, (2) DVE/Pool quantize W16->w8, (3) DVE/Pool subtract
W16-w8 -> dwq.
"""

import sys

for _p in ("/opt/trn_rl_repo", "/root/.axon_site/_ro/trn_rl_repo"):
    if _p not in sys.path:
        sys.path.append(_p)

import ml_dtypes
import numpy as np

import concourse.bass as bass
import concourse.mybir as mybir
from concourse import bacc
from concourse.bass_utils import run_bass_kernel_spmd
from concourse.tile import TileContext

B, C, O, KS, H, W, NUM = 32, 256, 256, 3, 28, 28, 8
P = 128
NCORES = 8
BG, OHALF = 4, 2          # batch-groups x out-channel halves
S = B // BG               # samples per core = 8
OC = O // OHALF           # out channels per core = 128
CC = C // P               # input-channel chunks = 2
G = 16                    # (k,o)-blocks per wgen matmul (with NUM=8 fills K=128)
NK = KS * KS              # 9 kernel positions

FW = W + 1                # flat row width (left zero col, right pad aliased)
NR = H + 2                # padded rows
FLAT = NR * FW + 2        # + guard zeros for the bottom-right window overrun
HO = H // 2               # 14 output rows per psum group
NF = HO * FW              # 406 psum columns per group (1 halo col per row)

NPROG = 3                 # samples whose conv groups run k-progressively

F32 = mybir.dt.float32
F16 = mybir.dt.float16
F8 = mybir.dt.float8e4
E4NP = ml_dtypes.float8_e4m3

_NC = None


def _build_nc():
    nc = bacc.Bacc()
    x8_d = nc.declare_dram_parameter("x8", [S, P, CC, FLAT], F8, isOutput=False)
    dx8_d = nc.declare_dram_parameter("dx8", [S, P, CC, FLAT], F8, isOutput=False)
    wp_d = nc.declare_dram_parameter("wp", [NK, P, CC, 8, P], F16, isOutput=False)
    se_d = nc.declare_dram_parameter("sebd", [P, P], F16, isOutput=False)
    b_d = nc.declare_dram_parameter("bias", [P, 1], F32, isOutput=False)
    out_d = nc.declare_dram_parameter("out", [S, P, H, W], F16, isOutput=True)

    with TileContext(nc) as tc:
        with (
            tc.tile_pool(name="constp", bufs=1) as constp,
            tc.tile_pool(name="wstream", bufs=4) as wstream,
            tc.tile_pool(name="xpool", bufs=1) as xpool,
            tc.tile_pool(name="slabp", bufs=1) as slabp,
            tc.tile_pool(name="outp", bufs=4) as outp,
            tc.tile_pool(name="wgps", bufs=2, space="PSUM") as wgps,
            tc.tile_pool(name="cvps", bufs=1, space="PSUM") as cvps,
        ):
            se_sb = constp.tile([P, P], F16)
            nc.sync.dma_start(out=se_sb, in_=se_d[:, :])
            bias_sb = constp.tile([P, 1], F32)
            nc.sync.dma_start(out=bias_sb, in_=b_d[:, :])

            # slabs: [c_lo, cc, k, s, oc]; conv lhsT = slab[:, :, k, s, :]
            w16 = slabp.tile([P, CC, NK, S, OC], F16)
            w8 = slabp.tile([P, CC, NK, S, OC], F8)
            dwq = slabp.tile([P, CC, NK, S, OC], F8)

            xts = [None] * S
            dxts = [None] * S

            def emit_xload(s):
                xts[s] = xpool.tile([P, CC, FLAT], F8, name=f"x8_{s}",
                                    tag=f"x8_{s}")
                nc.sync.dma_start(out=xts[s], in_=x8_d[s, :, :, :])
                dxts[s] = xpool.tile([P, CC, FLAT], F8, name=f"dx8_{s}",
                                     tag=f"dx8_{s}")
                nc.sync.dma_start(out=dxts[s], in_=dx8_d[s, :, :, :])

            def emit_wload(k):
                wt = wstream.tile([P, CC, 8, P], F16, name=f"wp_{k}", tag="wp")
                nc.sync.dma_start(out=wt, in_=wp_d[k, :, :, :, :])
                return wt

            # wp k=0,1 first so filter generation starts ASAP; the first
            # progressive samples' activations stream behind them.
            wts = {0: emit_wload(0)}
            emit_xload(0)
            wts[1] = emit_wload(1)
            emit_xload(1)
            emit_xload(2)

            _t1 = [0]

            def emit_wgen(cc, k, wt):
                # 8 matmuls -> 2 psum tiles; pass 1: copy to the fp16 W slab
                # (mostly Act; every 4th on DVE) so PSUM frees fast.
                for jg in range(2):
                    ps = wgps.tile([P, 4 * P], F32)
                    for i in range(4):
                        nc.tensor.matmul(
                            ps[:, i * P:(i + 1) * P], wt[:, cc, jg * 4 + i, :],
                            se_sb, start=True, stop=True,
                        )
                    src = ps.rearrange("p (i s g) -> p i s g", i=4, s=S, g=G)
                    dst = w16[:, cc, k, :, jg * 64:(jg + 1) * 64].rearrange(
                        "p s (i g) -> p i s g", g=G)
                    t = _t1[0]
                    _t1[0] += 1
                    if t % 4 == 3:
                        nc.vector.tensor_copy(out=dst, in_=src)
                    else:
                        nc.scalar.activation(
                            dst, src, mybir.ActivationFunctionType.Identity)

            def emit_pass23(cc, k):
                # pass 2: w8 = fp8(W16); pass 3: dwq = fp8(W16 - w8).
                # SBUF->SBUF, split DVE/Pool by parity.
                wsrc = w16[:, cc, k].rearrange("p s o -> p (s o)")
                wdst = w8[:, cc, k].rearrange("p s o -> p (s o)")
                ddst = dwq[:, cc, k].rearrange("p s o -> p (s o)")
                if (2 * k + cc) % 2 == 0:
                    nc.vector.tensor_copy(out=wdst, in_=wsrc)
                    nc.gpsimd.tensor_tensor(
                        ddst, wsrc, wdst, mybir.AluOpType.subtract)
                else:
                    nc.gpsimd.tensor_copy(out=wdst, in_=wsrc)
                    nc.vector.tensor_tensor(
                        ddst, wsrc, wdst, mybir.AluOpType.subtract)

            def emit_conv_term(k, s, hi, pst, ti, first=False, last=False):
                ky, kx = k // KS, k % KS
                st = (hi * HO + ky) * FW + kx
                stat, mov = ((w8, xts[s]), (w8, dxts[s]), (dwq, xts[s]))[ti]
                nc.tensor.matmul(
                    pst, stat[:, :, k, s, :], mov[:, :, st:st + NF],
                    start=first, stop=last,
                    perf_mode=mybir.MatmulPerfMode.DoubleRow,
                    skip_group_check=True,
                )

            outts = [None] * S

            def emit_evac(s, hi, pst):
                if hi == 0:
                    outts[s] = outp.tile([P, 2, HO, W], F16, name=f"ot_{s}",
                                         tag="ot")
                nc.scalar.activation(
                    outts[s][:, hi],
                    pst.rearrange("p (h w) -> p h w", w=FW)[:, :, 0:W],
                    mybir.ActivationFunctionType.Identity,
                    bias=bias_sb[:, 0:1], scale=1.0 / 16.0,
                )
                if hi == 1:
                    nc.sync.dma_start(
                        out=out_d[s, :, :, :],
                        in_=outts[s].rearrange("p t h w -> p (t h) w"),
                    )

            # progressive phase: filter-generation k-steps interleaved with
            # conv matmuls (main/dx one k behind, dw two) for NPROG samples.
            prog = {
                (s, hi): cvps.tile([P, NF], F32, name=f"pg_{s}_{hi}",
                                   tag=f"cv_{(s * 2 + hi) % (2 * NPROG)}")
                for s in range(NPROG) for hi in range(2)
            }
            for k in range(NK):
                emit_wgen(0, k, wts[k])
                emit_wgen(1, k, wts[k])
                emit_pass23(0, k)
                emit_pass23(1, k)
                if k + 2 < NK:
                    wts[k + 2] = emit_wload(k + 2)
                elif k + 2 == NK:
                    for s in range(NPROG, S):
                        emit_xload(s)
                if k >= 1:
                    for s in range(NPROG):
                        for hi in range(2):
                            emit_conv_term(k - 1, s, hi, prog[(s, hi)], 0,
                                           first=(k == 1))
                            emit_conv_term(k - 1, s, hi, prog[(s, hi)], 1)
                if k >= 2:
                    for s in range(NPROG):
                        for hi in range(2):
                            emit_conv_term(k - 2, s, hi, prog[(s, hi)], 2)
            for s in range(NPROG):
                for hi in range(2):
                    pst = prog[(s, hi)]
                    emit_conv_term(NK - 1, s, hi, pst, 0)
                    emit_conv_term(NK - 1, s, hi, pst, 1)
                    emit_conv_term(NK - 2, s, hi, pst, 2)
                    emit_conv_term(NK - 1, s, hi, pst, 2, last=True)
                    emit_evac(s, hi, pst)

            # burst phase: one 27-matmul group per remaining (sample, half).
            for s in range(NPROG, S):
                for hi in range(2):
                    pst = cvps.tile([P, NF], F32, name=f"bt_{s}_{hi}",
                                    tag=f"cv_{(s * 2 + hi) % (2 * NPROG)}")
                    for k in range(NK):
                        for ti in range(3):
                            emit_conv_term(k, s, hi, pst, ti,
                                           first=(k == 0 and ti == 0),
                                           last=(k == NK - 1 and ti == 2))
                    emit_evac(s, hi, pst)

    nc.compile()
    return nc


def _get_nc():
    global _NC
    if _NC is None:
        _NC = _build_nc()
    return _NC


def _prep_core_inputs(inputs, inputs_se, weight, bias, bg, oh):
    # weight rows: r = o*(C*9) + c*9 + (ky*3+kx)  -> [O, C, 3, 3, NUM]
    wr = weight.reshape(O, C, KS, KS, NUM)
    wo = wr[oh * OC:(oh + 1) * OC].reshape(OC, C, NK, NUM)  # [o, c, k, n]
    # [j, g, cc, c_lo, k, n] -> [k, n, g, cc, j, c_lo]; p = n*16+g
    t = wo.reshape(8, G, CC, P, NK, NUM)
    wp = t.transpose(4, 5, 1, 2, 0, 3).reshape(NK, P, CC, 8, P)
    wp = np.ascontiguousarray(wp.astype(np.float16))

    # block-diagonal 16*se: [(n,g), (s,g')] nonzero iff g==g'
    se16 = (16.0 * inputs_se[bg * S:(bg + 1) * S]).astype(np.float32)  # [s, n]
    sebd = np.zeros((NUM, G, S, G), dtype=np.float32)
    for g in range(G):
        sebd[:, g, :, g] = se16.T
    sebd = sebd.reshape(P, P).astype(np.float16)

    # activations: fp8 split, width-29 row-flat layout with guard zeros
    x_core = inputs[bg * S:(bg + 1) * S].astype(np.float32)
    x8 = x_core.astype(E4NP)
    dx8 = (x_core - x8.astype(np.float32)).astype(E4NP)

    def to_flat(a):
        f = np.zeros((S, CC, P, NR, FW), dtype=E4NP)
        f[:, :, :, 1:H + 1, 1:W + 1] = a.reshape(S, CC, P, H, W)
        out = np.zeros((S, CC, P, FLAT), dtype=E4NP)
        out[:, :, :, :NR * FW] = f.reshape(S, CC, P, NR * FW)
        return np.ascontiguousarray(out.transpose(0, 2, 1, 3))

    return {
        "x8": to_flat(x8),
        "dx8": to_flat(dx8),
        "wp": wp,
        "sebd": sebd,
        "bias": np.ascontiguousarray(
            bias[oh * OC:(oh + 1) * OC].reshape(OC, 1), dtype=np.float32
        ),
    }


def kernel(inputs, inputs_se, weight, bias):
    inputs = np.asarray(inputs, dtype=np.float32)
    inputs_se = np.asarray(inputs_se, dtype=np.float32)
    weight = np.asarray(weight, dtype=np.float32)
    bias = np.asarray(bias, dtype=np.float32)

    nc = _get_nc()
    in_maps = []
    for core in range(NCORES):
        bg, oh = core // OHALF, core % OHALF
        in_maps.append(_prep_core_inputs(inputs, inputs_se, weight, bias, bg, oh))

    res = run_bass_kernel_spmd(nc, in_maps, list(range(NCORES))).results

    out = np.empty((B, O, H, W), dtype=np.float32)
    for core in range(NCORES):
        bg, oh = core // OHALF, core % OHALF
        out[bg * S:(bg + 1) * S, oh * OC:(oh + 1) * OC] = (
            res[core]["out"].astype(np.float32))
    return out


# revision 10
# speedup vs baseline: 1.1024x; 1.1024x over previous
"""Trainium2 Bass kernel for nn_DiverseRegDCConv2d.

Per-sample dynamic 3x3 conv: filters are generated per sample from an
8-column weight bank (wgen[b] = se[b] @ bank.T), then applied as a
standard 256->256 conv on 28x28 with padding 1.

Sharding (8 cores): 4 batch-groups x 2 out-channel halves. Each core
handles 8 samples x 128 out channels; the weight bank half it needs is
replicated across the 4 batch-groups. No cross-device communication.

Numerics: the conv runs on fp8e4 (e4m3) operands in DoubleRow perf
mode (K=256 per matmul, 0.5 cycles per output column) with a 3-term
residual split that recovers ~fp16 accuracy:

    out = (w8 (.) x8  +  w8 (.) dx8  +  dwq (.) x8) / 16

where the filters are generated on device at 16x scale (se pre-scaled
on host), w8 = fp8(W), dwq = fp8(W - w8), and the activations are
split on host as x8 = fp8(x), dx8 = fp8(x - x8). End-to-end relative
error vs the fp32 reference is ~1.5e-3.

Conv layout trick: activations are stored width-29 row-flattened
(left zero-pad column only; the right pad of row r aliases the left
zero of row r+1), so every 3x3 shifted window is a single contiguous
406-element run and the DoubleRow moving AP stays 3-D. Each PSUM row
has one discarded halo column (29 vs 28).

Filter generation stays on device: the bank half is pre-arranged on
the host into 128x128 stationary tiles whose partition axis is (n, g)
with n = bank column (8) and g = 16 (k,o)-blocks; the streaming
operand is a block-diagonal arrangement of 16*inputs_se. Evacuation is
split into three passes so PSUM turns around fast and the slow fp8
ops run SBUF->SBUF where GPSIMD can help: (1) Act/DVE copy PSUM->W16
scratch, (2) DVE/Pool quantize W16->w8, (3) DVE/Pool subtract
W16-w8 -> dwq.
"""

import sys

for _p in ("/opt/trn_rl_repo", "/root/.axon_site/_ro/trn_rl_repo"):
    if _p not in sys.path:
        sys.path.append(_p)

import ml_dtypes
import numpy as np

import concourse.bass as bass
import concourse.mybir as mybir
from concourse import bacc
from concourse.bass_utils import run_bass_kernel_spmd
from concourse.tile import TileContext

B, C, O, KS, H, W, NUM = 32, 256, 256, 3, 28, 28, 8
P = 128
NCORES = 8
BG, OHALF = 4, 2          # batch-groups x out-channel halves
S = B // BG               # samples per core = 8
OC = O // OHALF           # out channels per core = 128
CC = C // P               # input-channel chunks = 2
G = 16                    # (k,o)-blocks per wgen matmul (with NUM=8 fills K=128)
NK = KS * KS              # 9 kernel positions

FW = W + 1                # flat row width (left zero col, right pad aliased)
NR = H + 2                # padded rows
FLAT = NR * FW + 2        # + guard zeros for the bottom-right window overrun
HO = H // 2               # 14 output rows per psum group
NF = HO * FW              # 406 psum columns per group (1 halo col per row)

NPROG = 3                 # samples whose conv groups run k-progressively

F32 = mybir.dt.float32
F16 = mybir.dt.float16
F8 = mybir.dt.float8e4
E4NP = ml_dtypes.float8_e4m3

_NC = None


def _build_nc():
    nc = bacc.Bacc()
    x8_d = nc.declare_dram_parameter("x8", [S, P, CC, FLAT], F8, isOutput=False)
    dx8_d = nc.declare_dram_parameter("dx8", [S, P, CC, FLAT], F8, isOutput=False)
    wp_d = nc.declare_dram_parameter("wp", [NK, P, CC, 8, P], F16, isOutput=False)
    se_d = nc.declare_dram_parameter("sebd", [P, P], F16, isOutput=False)
    b_d = nc.declare_dram_parameter("bias", [P, 1], F32, isOutput=False)
    out_d = nc.declare_dram_parameter("out", [S, P, H, W], F16, isOutput=True)

    with TileContext(nc) as tc:
        with (
            tc.tile_pool(name="constp", bufs=1) as constp,
            tc.tile_pool(name="wstream", bufs=4) as wstream,
            tc.tile_pool(name="xpool", bufs=1) as xpool,
            tc.tile_pool(name="slabp", bufs=1) as slabp,
            tc.tile_pool(name="outp", bufs=4) as outp,
            tc.tile_pool(name="wgps", bufs=2, space="PSUM") as wgps,
            tc.tile_pool(name="cvps", bufs=1, space="PSUM") as cvps,
        ):
            # slabs: [c_lo, cc, k, s, oc]; conv lhsT = slab[:, :, k, s, :]
            w16 = slabp.tile([P, CC, NK, S, OC], F16)
            w8 = slabp.tile([P, CC, NK, S, OC], F8)
            dwq = slabp.tile([P, CC, NK, S, OC], F8)

            xts = [None] * S
            dxts = [None] * S

            def emit_xload(s):
                xts[s] = xpool.tile([P, CC, FLAT], F8, name=f"x8_{s}",
                                    tag=f"x8_{s}")
                nc.sync.dma_start(out=xts[s], in_=x8_d[s, :, :, :])
                dxts[s] = xpool.tile([P, CC, FLAT], F8, name=f"dx8_{s}",
                                     tag=f"dx8_{s}")
                nc.sync.dma_start(out=dxts[s], in_=dx8_d[s, :, :, :])

            def emit_wload(k, split=False):
                wt = wstream.tile([P, CC, 8, P], F16, name=f"wp_{k}", tag="wp")
                if split:
                    # two half-loads so wgen(cc=0, k) starts one transfer early
                    nc.sync.dma_start(out=wt[:, 0], in_=wp_d[k, :, 0, :, :])
                    nc.sync.dma_start(out=wt[:, 1], in_=wp_d[k, :, 1, :, :])
                else:
                    nc.sync.dma_start(out=wt, in_=wp_d[k, :, :, :, :])
                return wt

            # wp k=0 cc=0 leads the DMA queue so filter generation starts
            # ASAP; se/bias slot into its shadow, then the progressive
            # samples' activations stream behind wp k=1.
            wts = {}
            wt0 = wstream.tile([P, CC, 8, P], F16, name="wp_0", tag="wp")
            nc.sync.dma_start(out=wt0[:, 0], in_=wp_d[0, :, 0, :, :])
            se_sb = constp.tile([P, P], F16)
            nc.sync.dma_start(out=se_sb, in_=se_d[:, :])
            bias_sb = constp.tile([P, 1], F32)
            nc.sync.dma_start(out=bias_sb, in_=b_d[:, :])
            nc.sync.dma_start(out=wt0[:, 1], in_=wp_d[0, :, 1, :, :])
            wts[0] = wt0
            wts[1] = emit_wload(1)
            emit_xload(0)
            emit_xload(1)
            emit_xload(2)

            def emit_wgen(cc, k, wt):
                # 8 matmuls -> 2 psum tiles; pass 1 (Act): copy to the fp16
                # W slab so PSUM frees fast.
                for jg in range(2):
                    ps = wgps.tile([P, 4 * P], F32)
                    for i in range(4):
                        nc.tensor.matmul(
                            ps[:, i * P:(i + 1) * P], wt[:, cc, jg * 4 + i, :],
                            se_sb, start=True, stop=True,
                        )
                    src = ps.rearrange("p (i s g) -> p i s g", i=4, s=S, g=G)
                    dst = w16[:, cc, k, :, jg * 64:(jg + 1) * 64].rearrange(
                        "p s (i g) -> p i s g", g=G)
                    nc.scalar.activation(
                        dst, src, mybir.ActivationFunctionType.Identity)

            def emit_pass23(cc, k):
                # pass 2 (DVE, 2x all-SBUF mode): w8 = fp8(W16);
                # pass 3 (DVE/Pool alternating): dwq = fp8(W16 - w8).
                wsrc = w16[:, cc, k].rearrange("p s o -> p (s o)")
                wdst = w8[:, cc, k].rearrange("p s o -> p (s o)")
                ddst = dwq[:, cc, k].rearrange("p s o -> p (s o)")
                nc.vector.tensor_copy(out=wdst, in_=wsrc)
                eng = nc.vector if cc == 0 else nc.gpsimd
                eng.tensor_tensor(ddst, wsrc, wdst, mybir.AluOpType.subtract)

            def emit_conv_term(k, s, hi, pst, ti, first=False, last=False):
                ky, kx = k // KS, k % KS
                st = (hi * HO + ky) * FW + kx
                stat, mov = ((w8, xts[s]), (w8, dxts[s]), (dwq, xts[s]))[ti]
                nc.tensor.matmul(
                    pst, stat[:, :, k, s, :], mov[:, :, st:st + NF],
                    start=first, stop=last,
                    perf_mode=mybir.MatmulPerfMode.DoubleRow,
                    skip_group_check=True,
                )

            outts = [None] * S

            def emit_evac(s, hi, pst):
                if hi == 0:
                    outts[s] = outp.tile([P, 2, HO, W], F16, name=f"ot_{s}",
                                         tag="ot")
                nc.scalar.activation(
                    outts[s][:, hi],
                    pst.rearrange("p (h w) -> p h w", w=FW)[:, :, 0:W],
                    mybir.ActivationFunctionType.Identity,
                    bias=bias_sb[:, 0:1], scale=1.0 / 16.0,
                )
                # per-half store so the final group's DMA tail is short
                nc.sync.dma_start(
                    out=out_d[s, :, hi * HO:(hi + 1) * HO, :],
                    in_=outts[s][:, hi],
                )

            # progressive phase: filter-generation k-steps interleaved with
            # conv matmuls (main/dx one k behind, dw two) for NPROG samples.
            prog = {
                (s, hi): cvps.tile([P, NF], F32, name=f"pg_{s}_{hi}",
                                   tag=f"cv_{(s * 2 + hi) % (2 * NPROG)}")
                for s in range(NPROG) for hi in range(2)
            }
            for k in range(NK):
                emit_wgen(0, k, wts[k])
                emit_wgen(1, k, wts[k])
                emit_pass23(0, k)
                emit_pass23(1, k)
                if k + 2 < NK:
                    wts[k + 2] = emit_wload(k + 2)
                if k >= 4:
                    emit_xload(NPROG + (k - 4))
                if k >= 1:
                    for s in range(NPROG):
                        for hi in range(2):
                            emit_conv_term(k - 1, s, hi, prog[(s, hi)], 0,
                                           first=(k == 1))
                            emit_conv_term(k - 1, s, hi, prog[(s, hi)], 1)
                if k >= 2:
                    for s in range(NPROG):
                        for hi in range(2):
                            emit_conv_term(k - 2, s, hi, prog[(s, hi)], 2)
            for s in range(NPROG):
                for hi in range(2):
                    pst = prog[(s, hi)]
                    emit_conv_term(NK - 1, s, hi, pst, 0)
                    emit_conv_term(NK - 1, s, hi, pst, 1)
                    emit_conv_term(NK - 2, s, hi, pst, 2)
                    emit_conv_term(NK - 1, s, hi, pst, 2, last=True)
                    emit_evac(s, hi, pst)

            # burst phase: one 27-matmul group per remaining (sample, half).
            for s in range(NPROG, S):
                for hi in range(2):
                    pst = cvps.tile([P, NF], F32, name=f"bt_{s}_{hi}",
                                    tag=f"cv_{(s * 2 + hi) % (2 * NPROG)}")
                    for k in range(NK):
                        for ti in range(3):
                            emit_conv_term(k, s, hi, pst, ti,
                                           first=(k == 0 and ti == 0),
                                           last=(k == NK - 1 and ti == 2))
                    emit_evac(s, hi, pst)

    nc.compile()
    return nc


def _get_nc():
    global _NC
    if _NC is None:
        _NC = _build_nc()
    return _NC


def _prep_core_inputs(inputs, inputs_se, weight, bias, bg, oh):
    # weight rows: r = o*(C*9) + c*9 + (ky*3+kx)  -> [O, C, 3, 3, NUM]
    wr = weight.reshape(O, C, KS, KS, NUM)
    wo = wr[oh * OC:(oh + 1) * OC].reshape(OC, C, NK, NUM)  # [o, c, k, n]
    # [j, g, cc, c_lo, k, n] -> [k, n, g, cc, j, c_lo]; p = n*16+g
    t = wo.reshape(8, G, CC, P, NK, NUM)
    wp = t.transpose(4, 5, 1, 2, 0, 3).reshape(NK, P, CC, 8, P)
    wp = np.ascontiguousarray(wp.astype(np.float16))

    # block-diagonal 16*se: [(n,g), (s,g')] nonzero iff g==g'
    se16 = (16.0 * inputs_se[bg * S:(bg + 1) * S]).astype(np.float32)  # [s, n]
    sebd = np.zeros((NUM, G, S, G), dtype=np.float32)
    for g in range(G):
        sebd[:, g, :, g] = se16.T
    sebd = sebd.reshape(P, P).astype(np.float16)

    # activations: fp8 split, width-29 row-flat layout with guard zeros
    x_core = inputs[bg * S:(bg + 1) * S].astype(np.float32)
    x8 = x_core.astype(E4NP)
    dx8 = (x_core - x8.astype(np.float32)).astype(E4NP)

    def to_flat(a):
        f = np.zeros((S, CC, P, NR, FW), dtype=E4NP)
        f[:, :, :, 1:H + 1, 1:W + 1] = a.reshape(S, CC, P, H, W)
        out = np.zeros((S, CC, P, FLAT), dtype=E4NP)
        out[:, :, :, :NR * FW] = f.reshape(S, CC, P, NR * FW)
        return np.ascontiguousarray(out.transpose(0, 2, 1, 3))

    return {
        "x8": to_flat(x8),
        "dx8": to_flat(dx8),
        "wp": wp,
        "sebd": sebd,
        "bias": np.ascontiguousarray(
            bias[oh * OC:(oh + 1) * OC].reshape(OC, 1), dtype=np.float32
        ),
    }


def kernel(inputs, inputs_se, weight, bias):
    inputs = np.asarray(inputs, dtype=np.float32)
    inputs_se = np.asarray(inputs_se, dtype=np.float32)
    weight = np.asarray(weight, dtype=np.float32)
    bias = np.asarray(bias, dtype=np.float32)

    nc = _get_nc()
    in_maps = []
    for core in range(NCORES):
        bg, oh = core // OHALF, core % OHALF
        in_maps.append(_prep_core_inputs(inputs, inputs_se, weight, bias, bg, oh))

    res = run_bass_kernel_spmd(nc, in_maps, list(range(NCORES))).results

    out = np.empty((B, O, H, W), dtype=np.float32)
    for core in range(NCORES):
        bg, oh = core // OHALF, core % OHALF
        out[bg * S:(bg + 1) * S, oh * OC:(oh + 1) * OC] = (
            res[core]["out"].astype(np.float32))
    return out
